# revision 28
# baseline (speedup 1.0000x reference)
"""Trainium2 Bass kernel for Bahdanau-style attention (8 NeuronCores, data-parallel).

Reference computation (per batch b):
    h = hidden[-1]                       # [B, H]
    u = h @ U_w.T + U_b                  # [B, H]
    w = enc @ W_w.T + W_b                # [B, S, H]
    comb = tanh(u[:, None, :] + w)       # [B, S, H]
    energy = comb @ V_w[0] + V_b[0]      # [B, S]
    attn = softmax(energy, axis=1)       # [B, S]
    context = attn @ enc                 # [B, H]
    return (context, attn)

Sharding: batch dim B=64 across 8 cores (8 batches/core); params replicated.

Per-core device algorithm (all bulk compute in bf16, fp32 accumulation):
  - z8[b,k] = h_b @ U_w.T + U_b + W_b          (prologue, one matmul group)
  - per batch, per s-tile pair: psum[s,k] = z (via batch-select rank-8 matmul)
      + sum_h encT[h,s] * Wt[h,k]  (TensorE), tanh on ScalarE -> bf16,
      energy[s] = sum_k comb*V on VectorE scalar_tensor_tensor with accum_out
      (per-s_tile columns of energy land in [128, 16] f32, s on partitions;
      V_b is dropped entirely -- softmax is invariant to a constant shift)
  - p = exp(energy) (no max subtraction: |energy| < ~23 worst case, exp safe in f32)
  - denom via two tiny matmuls with ones vectors; inv = reciprocal
  - attn = p * inv (per-partition scalar broadcast via rank-1 matmul)
  - context = sum_t p_col[t] @ enc_nat[t]  (TensorE, deferred one batch to keep
    the PE busy while VectorE/ScalarE finish the softmax chain of the previous batch)
"""

import sys

import numpy as np

if "/opt/trn_rl_repo" not in sys.path:
    sys.path.insert(0, "/opt/trn_rl_repo")

import ml_dtypes

import concourse.bass as bass
import concourse.tile as tile
from concourse import bacc, mybir
from concourse.bass import ts

B, S, H = 64, 2048, 512
NCORES = 8
BL = B // NCORES          # 8 batches per core
PT = 128                  # partition tile
NH = H // PT              # 4 h-chunks
NT = S // PT              # 16 s-tiles
KT = H                    # 512 output (k) free size

F32 = mybir.dt.float32
BF16 = mybir.dt.bfloat16
BF = ml_dtypes.bfloat16

_CACHE = {}


def _build(repeat=1):
    """Build + compile the per-core Bass program (identical on all cores).

    repeat>1 replicates the whole batch pipeline (same data) for wall-clock
    delta benchmarking; outputs are overwritten identically each repetition."""
    nc = bacc.Bacc(None, target_bir_lowering=False)
    Tanh = mybir.ActivationFunctionType.Tanh
    Exp = mybir.ActivationFunctionType.Exp

    encT_d = nc.dram_tensor("encT", [BL, 4, PT, NH, S // 4], BF16, kind="ExternalInput")
    encN_d = nc.dram_tensor("encN", [BL, 4, PT, NT // 4, H], BF16, kind="ExternalInput")
    hT_d = nc.dram_tensor("hT", [PT, NH, BL], BF16, kind="ExternalInput")
    Wt_d = nc.dram_tensor("Wt", [PT, NH, KT], BF16, kind="ExternalInput")
    Ut_d = nc.dram_tensor("Ut", [PT, NH, KT], BF16, kind="ExternalInput")
    Vrep_d = nc.dram_tensor("Vrep", [PT, KT], BF16, kind="ExternalInput")
    bsel_d = nc.dram_tensor("bsel", [BL, BL, PT], BF16, kind="ExternalInput")
    Ub_d = nc.dram_tensor("Ub", [1, KT], BF16, kind="ExternalInput")
    Wb_d = nc.dram_tensor("Wb", [1, KT], BF16, kind="ExternalInput")
    out_d = nc.dram_tensor("out", [BL, S + KT], F32, kind="ExternalOutput")

    with tile.TileContext(nc) as tc:
        with (
            tc.tile_pool(name="const", bufs=1) as constp,
            tc.tile_pool(name="encT", bufs=10) as encTp,
            tc.tile_pool(name="encN", bufs=12) as encNp,
            tc.tile_pool(name="comb", bufs=3) as combp,
            tc.tile_pool(name="dead", bufs=2) as deadp,
            tc.tile_pool(name="small", bufs=3) as smallp,
            tc.tile_pool(name="wp", bufs=2, space="PSUM") as wpp,
            tc.tile_pool(name="pctx", bufs=2, space="PSUM") as ctxp,
            tc.tile_pool(name="psmall", bufs=2, space="PSUM") as psmallp,
        ):
            # encoder tiles split into s-quarters: the first matmul pair only
            # waits on the first quarter's DMA; quarters for batch b+1 are
            # prefetched during batch b.  Batch 0's quarters are issued before
            # every constant so the PE's first W-matmul starts ASAP.
            def load_encT(b, qrange=range(4), qs=None):
                qs = qs if qs is not None else []
                for q in qrange:
                    tq = encTp.tile([PT, NH, S // 4], BF16, tag="encT")
                    nc.sync.dma_start(out=tq, in_=encT_d[b, q])
                    qs.append(tq)
                return qs


            # ---------- constants ----------
            Wt_sb = constp.tile([PT, NH, KT], BF16)
            nc.sync.dma_start(out=Wt_sb, in_=Wt_d[:])
            Ut_sb = constp.tile([PT, NH, KT], BF16)
            nc.sync.dma_start(out=Ut_sb, in_=Ut_d[:])
            hT_sb = constp.tile([PT, NH, BL], BF16)
            nc.sync.dma_start(out=hT_sb, in_=hT_d[:])
            Ub_sb = constp.tile([1, KT], BF16)
            nc.sync.dma_start(out=Ub_sb, in_=Ub_d[:])
            Wb_sb = constp.tile([1, KT], BF16)
            nc.sync.dma_start(out=Wb_sb, in_=Wb_d[:])
            # one-hot batch-select constant (tiny, needed by pair-0's z matmul)
            bsel_sb = constp.tile([BL, BL, PT], BF16)
            nc.sync.dma_start(out=bsel_sb, in_=bsel_d[:])
            Vrep_sb = constp.tile([PT, KT], BF16)
            nc.sync.dma_start(out=Vrep_sb, in_=Vrep_d[:])
            ones128f_sb = constp.tile([PT, 1], F32)
            nc.vector.memset(ones128f_sb, 1.0)
            onesr_bf_sb = constp.tile([1, PT], BF16)
            nc.vector.memset(onesr_bf_sb, 1.0)
            onesr_f32_sb = constp.tile([1, PT], F32)
            nc.vector.memset(onesr_f32_sb, 1.0)
            # standalone bf16 ldweights: absorb the DMA-lane semaphore ticks
            # of PE-read constants into the PE's observed vector clock, so the
            # matmuls that follow each need at most one sync wait.
            nc.tensor.ldweights(Wt_sb[:, 0, 0:PT])

            # ---------- z8 = h @ U_w.T + U_b + W_b  -> [BL, KT] bf16 ----------
            # Each accumulation-group-starting matmul on a recycled PSUM tile
            # would otherwise carry 2 sync waits (WAR vs ScalarE/VectorE reader
            # + WAW vs earlier PE writes) -- the Matmult ISA struct only has
            # one wait slot (walrus: "Too many sync wait commands").  A tiny
            # VectorE memset "claims" the tile first: it absorbs the
            # cross-engine waits (DVE ops take up to 3) and the matmul then
            # only waits on the memset.
            def claim(ps):
                nc.vector.memset(ps[0:1, ..., 0:1] if ps.ndim > 2 else ps[0:1, 0:1], 0.0)

            z8_sb = constp.tile([BL, KT], BF16)

            def emit_z8():
                # emitted between pair-0's W-matmuls and its z-matmul so the
                # PE starts on encoder data (Wt + first quarter) immediately
                # instead of stalling on the Ut/hT const DMAs.
                nc.tensor.ldweights(Ut_sb[:, 0, 0:PT])
                nc.tensor.ldweights(hT_sb[:, 0, :])
                nc.tensor.ldweights(bsel_sb[:, 0, :])
                z8_ps = ctxp.tile([BL, KT], F32, tag="ctx")
                claim(z8_ps)
                for hc in range(NH):
                    nc.tensor.matmul(
                        z8_ps, hT_sb[:, hc, :], Ut_sb[:, hc, :],
                        start=(hc == 0), stop=False,
                    )
                nc.tensor.matmul(
                    z8_ps, onesr_bf_sb[:, :BL], Ub_sb, start=False, stop=False)
                nc.tensor.matmul(
                    z8_ps, onesr_bf_sb[:, :BL], Wb_sb, start=False, stop=True)
                nc.vector.tensor_copy(z8_sb, z8_ps)

            # ---------- main per-batch pipeline ----------
            # wp PSUM tiles are allocated + claimed two pairs ahead so the
            # tiny claim-memset sits in the VectorE FIFO before the V-dot ops
            # of the previous pairs (else the claim -- and with it the next
            # pair's matmuls -- waits behind ~1.3us of V-dot work).
            pending_wp = []

            def alloc_wp():
                t = wpp.tile([PT, 2, KT], F32, tag="wp")
                nc.vector.memset(t[0:1, :, 0:1], 0.0)
                pending_wp.append(t)

            alloc_wp()
            alloc_wp()
            for _rep in range(repeat):
              deferred = [None] * BL
              for b in range(BL + 1):
                if b < BL:
                    if b == 0:
                        encT_next = load_encT(0)
                    encT_q = encT_next
                    if b + 1 < BL:
                        encT_next = load_encT(b + 1)
                    energy_sb = smallp.tile([PT, NT], F32, tag="energy")
                    for pair in range(NT // 2):
                        t0 = 2 * pair
                        wp_t = pending_wp.pop(0)
                        # encT.T @ Wt, then the z-broadcast (batch-select) matmul
                        # last -- lets the first W-matmuls start before z8 is ready
                        for j in (0, 1):
                            t = t0 + j
                            for hc in range(NH):
                                nc.tensor.matmul(
                                    wp_t[:, j, :],
                                    encT_q[t // 4][:, hc, ts(t % 4, PT)],
                                    Wt_sb[:, hc, :],
                                    start=(hc == 0), stop=False,
                                )
                        if _rep == 0 and b == 0 and pair == 0:
                            emit_z8()
                        for j in (0, 1):
                            nc.tensor.matmul(
                                wp_t[:, j, :], bsel_sb[:, b, :], z8_sb,
                                start=False, stop=True,
                            )
                        comb_sb = combp.tile([PT, 2, KT], BF16, tag="comb")
                        nc.scalar.activation(comb_sb, wp_t, Tanh)
                        alloc_wp()
                        for j in (0, 1):
                            dead_sb = deadp.tile([PT, KT], BF16, tag="dead")
                            nc.vector.scalar_tensor_tensor(
                                out=dead_sb,
                                in0=comb_sb[:, j, :],
                                scalar=1.0,
                                in1=Vrep_sb,
                                op0=mybir.AluOpType.mult,
                                op1=mybir.AluOpType.mult,
                                accum_out=energy_sb[:, t0 + j : t0 + j + 1],
                            )
                    # softmax pieces (denominator via ones-matmuls)
                    p_sb = smallp.tile([PT, NT], F32, tag="p")
                    nc.scalar.activation(p_sb, energy_sb, Exp)
                    pcols_sb = smallp.tile([PT, NT], BF16, tag="pcols")
                    nc.vector.tensor_copy(pcols_sb, p_sb)
                    rowsum_sb = smallp.tile([PT, 1], F32, tag="rowsum")
                    nc.vector.tensor_reduce(
                        rowsum_sb, p_sb, axis=mybir.AxisListType.X,
                        op=mybir.AluOpType.add,
                    )
                    ps1 = psmallp.tile([1, 1], F32, tag="ps")
                    nc.vector.memset(ps1[0:1, 0:1], 0.0)
                    nc.tensor.matmul(ps1, rowsum_sb, ones128f_sb, start=True, stop=True)
                    inv_sb = smallp.tile([1, 1], F32, tag="inv")
                    nc.vector.reciprocal(inv_sb, ps1)
                    invb_ps = psmallp.tile([PT, 1], F32, tag="ps")
                    nc.vector.memset(invb_ps[0:1, 0:1], 0.0)
                    nc.tensor.matmul(invb_ps, onesr_f32_sb, inv_sb, start=True, stop=True)
                    attn_sb = smallp.tile([PT, NT], F32, tag="attn")
                    nc.vector.tensor_scalar_mul(attn_sb, p_sb, invb_ps)
                    encN_q = []
                    for q in range(4):
                        tq = encNp.tile([PT, NT // 4, H], BF16, tag="encN")
                        nc.sync.dma_start(out=tq, in_=encN_d[b, q])
                        encN_q.append(tq)
                    nc.sync.dma_start(
                        out=out_d[b : b + 1, 0:S].rearrange("o (i t) -> (o i) t", i=PT),
                        in_=attn_sb,
                    )
                    deferred[b] = (encN_q, pcols_sb, inv_sb)
                if b >= 1:
                    encN_q, pcols_sb, inv_sb = deferred[b - 1]
                    for q in range(4):
                        nc.tensor.ldweights(encN_q[q][:, 0, 0:PT])
                    ctx_ps = ctxp.tile([1, KT], F32, tag="ctx")
                    nc.vector.memset(ctx_ps[0:1, 0:1], 0.0)
                    for t in range(NT):
                        nc.tensor.matmul(
                            ctx_ps, pcols_sb[:, t : t + 1],
                            encN_q[t // 4][:, t % 4, :],
                            start=(t == 0), stop=(t == NT - 1),
                        )
                    ctxr_sb = smallp.tile([1, KT], F32, tag="ctxr")
                    nc.vector.tensor_scalar_mul(ctxr_sb, ctx_ps, inv_sb)
                    nc.sync.dma_start(out=out_d[b - 1 : b, S : S + KT], in_=ctxr_sb)

    nc.compile()
    return nc


def _in_maps(hidden, encoder_outputs, U_w, U_b, W_w, W_b, V_w, V_b):
    """Shard + marshal full f32 inputs into per-core DRAM parameter dicts."""
    h = np.asarray(hidden, np.float32)[-1]                 # [B, H]
    enc = np.ascontiguousarray(np.asarray(encoder_outputs, np.float32))
    enc_bf = enc.astype(BF)                                # [B, S, H]
    # partition-major layouts so every DMA descriptor is a large contiguous
    # run per SBUF partition:
    #   encN[b, p, t, h] = enc[b, 128 t + p, h]
    encN_bf = np.ascontiguousarray(
        enc_bf.reshape(B, 4, NT // 4, PT, H).transpose(0, 1, 3, 2, 4)
    )
    #   encT[b, p, c, s] = enc[b, s, 128 c + p]
    encT_bf = np.ascontiguousarray(
        enc_bf.transpose(0, 2, 1).reshape(B, NH, PT, 4, S // 4)
        .transpose(0, 3, 2, 1, 4)
    )

    def pmajor(m):  # [H, K] -> [PT, NH, K] with row (c, p) -> [p, c]
        return np.ascontiguousarray(m.reshape(NH, PT, KT).transpose(1, 0, 2))

    Wt = pmajor(np.ascontiguousarray(np.asarray(W_w, np.float32).T).astype(BF))
    Ut = pmajor(np.ascontiguousarray(np.asarray(U_w, np.float32).T).astype(BF))
    Vrep = np.broadcast_to(np.asarray(V_w, np.float32)[0:1, :], (PT, KT))
    Vrep = np.ascontiguousarray(Vrep).astype(BF)
    bsel = np.zeros((BL, BL, PT), np.float32)
    for b in range(BL):
        bsel[b, b, :] = 1.0
    bsel = bsel.astype(BF)
    Ub = np.asarray(U_b, np.float32).reshape(1, KT).astype(BF)
    Wb = np.asarray(W_b, np.float32).reshape(1, KT).astype(BF)

    maps = []
    for c in range(NCORES):
        sl = slice(c * BL, (c + 1) * BL)
        hT = np.ascontiguousarray(h[sl].T).astype(BF)      # [H, BL]
        hT = np.ascontiguousarray(hT.reshape(NH, PT, BL).transpose(1, 0, 2))
        maps.append(
            {
                "encT": np.ascontiguousarray(encT_bf[sl]),
                "encN": np.ascontiguousarray(encN_bf[sl]),
                "hT": hT,
                "Wt": Wt,
                "Ut": Ut,
                "Vrep": Vrep,
                "bsel": bsel,
                "Ub": Ub,
                "Wb": Wb,
            }
        )
    return maps


def _run(inputs, trace=False, **kw):
    from concourse.bass_utils import run_bass_kernel_spmd

    if "nc" not in _CACHE:
        _CACHE["nc"] = _build()
    nc = _CACHE["nc"]
    maps = _in_maps(**inputs)
    res = run_bass_kernel_spmd(
        nc, maps, core_ids=list(range(NCORES)), trace=trace, **kw
    )
    outs = [r["out"] for r in res.results]                 # each [BL, S+KT] f32
    full = np.concatenate(outs, axis=0)                    # [B, S+KT]
    attn_it = full[:, :S].reshape(B, PT, NT)               # [b, i, t]; s = 128 t + i
    attn = np.ascontiguousarray(attn_it.transpose(0, 2, 1).reshape(B, S))
    context = np.ascontiguousarray(full[:, S:])
    return (context, attn), res


def kernel(**inputs):
    (context, attn), _ = _run(inputs, trace=False)
    return (context, attn)
